# revision 1
# baseline (speedup 1.0000x reference)
"""InternLM3 attention block on 8 Trainium2 NeuronCores (Bass/Tile), v2.

Strategy (tensor-parallel over heads, per the GQA structure):
  - 32 Q heads / 8 KV heads, head_dim 128.  Core c owns Q heads [4c,4c+4)
    and KV head c (one GQA group per core, so K/V never needs replication).
  - Per core, fused pipeline over 512-token blocks: QKV projection (fp32r
    matmuls, host-pretiled [p, ko, t] layouts so every DMA line is >=8KB
    contiguous) -> RoPE (2 DVE mults off PSUM + PE rotation matmul + DVE
    add) -> causal flash-style attention in S^T orientation:
      scores^T = kT-tile.T @ q^T  (PSUM), causal mask added via an
      identity-matmul of a -60000 bias tile (exp -> exact 0), exp on ACT,
      denominator accumulated on PE via a ones-column matmul, PV in PSUM,
      normalize with DVE reciprocal + gpsimd partition_broadcast.
  - Attention outputs (attn^T, [512 hid-slice, tok]) are AllGathered across
    the 8 cores in 8 token-chunks (overlapped with compute), then each core
    computes its 512-column slice of the output projection (N=512 matmuls).
  - Host only shards/pre-tiles inputs and concatenates the 8 output slices.

All matmuls run as float32r (TF32-like, full PE rate, ~1e-4 relative error).
"""

import math
import os
import sys

if "/opt/trn_rl_repo" not in sys.path:
    sys.path.insert(0, "/opt/trn_rl_repo")

import ml_dtypes
import numpy as np

import concourse.bass as bass
import concourse.mybir as mybir
import concourse.tile as tile
from concourse import bacc
from concourse import bass_utils

# ---- problem constants (hardcoded per harness contract) ----
HIDDEN = 4096
N_HEADS = 32
N_KV_HEADS = 8
HEAD_DIM = 128
ROPE_THETA = 10000.0
B, S = 2, 2048
NCORES = 8

P = 128
TQ = 512                      # token block
NB = S // TQ                  # 4 blocks per batch
KT = HIDDEN // P              # 32 contraction tiles
QH = N_HEADS // NCORES        # 4 q-heads per core
HG = QH * HEAD_DIM            # 512 = head-group width per core
NCHUNK = B * NB               # 8 allgather chunks
NBLK = NCHUNK
TOK = B * S                   # 4096 tokens
KB = 4                        # k-tiles per x DMA (1MB chunks)
MASKVAL = -60000.0            # exp(score + MASKVAL) == 0 exactly

f32 = mybir.dt.float32
f32r = mybir.dt.float32r
bf16 = mybir.dt.bfloat16


def _build_module(with_collectives=True):
    nc = bacc.Bacc("TRN2", target_bir_lowering=False, debug=False,
                   num_devices=NCORES)
    nc._skip_collectives = not with_collectives

    xt = nc.dram_tensor("xt", [NBLK, P, KT, TQ], f32r,
                        kind="ExternalInput").ap()
    wqt = nc.dram_tensor("wqt", [P, KT, HG], f32r, kind="ExternalInput").ap()
    wkt = nc.dram_tensor("wkt", [P, KT, HEAD_DIM], f32r,
                         kind="ExternalInput").ap()
    wvt = nc.dram_tensor("wvt", [P, KT, HEAD_DIM], f32r,
                         kind="ExternalInput").ap()
    wot = nc.dram_tensor("wot", [P, KT, HG], bf16, kind="ExternalInput").ap()
    cosT = nc.dram_tensor("cosT", [P, S], f32, kind="ExternalInput").ap()
    sinrT = nc.dram_tensor("sinrT", [P, S], f32, kind="ExternalInput").ap()
    maskIn = nc.dram_tensor("maskIn", [P, 4 * TQ], f32,
                            kind="ExternalInput").ap()
    identIn = nc.dram_tensor("identIn", [P, P], f32, kind="ExternalInput").ap()
    protIn = nc.dram_tensor("protIn", [P, P], f32, kind="ExternalInput").ap()
    onesIn = nc.dram_tensor("onesIn", [P, 1], f32r, kind="ExternalInput").ap()
    outT = nc.dram_tensor("outT", [HG, TOK], f32, kind="ExternalOutput").ap()

    ag_in = [
        nc.dram_tensor(f"ag_in{i}", [HG, TQ], bf16, kind="Internal").ap()
        for i in range(NCHUNK)
    ]
    ag_out = [
        nc.dram_tensor(f"ag_out{i}", [HIDDEN, TQ], bf16, kind="Internal",
                       addr_space="Shared").ap()
        for i in range(NCHUNK)
    ]

    with tile.TileContext(nc) as tc:
        _body(tc, nc, xt, wqt, wkt, wvt, wot, cosT, sinrT, maskIn, identIn,
              protIn, onesIn, outT, ag_in, ag_out)
    nc.compile()
    return nc


def _body(tc, nc, xt, wqt, wkt, wvt, wot, cosT, sinrT, maskIn, identIn,
          protIn, onesIn, outT, ag_in, ag_out):
    AF = mybir.ActivationFunctionType
    OP = mybir.AluOpType

    with (
        tc.tile_pool(name="wpool", bufs=1) as wpool,
        tc.tile_pool(name="xpool", bufs=3) as xpool,
        tc.tile_pool(name="kvpool", bufs=1) as kvpool,
        tc.tile_pool(name="qpool", bufs=1) as qpool,
        tc.tile_pool(name="rtmp", bufs=2) as rtmp,
        tc.tile_pool(name="epool", bufs=6) as epool,
        tc.tile_pool(name="aux", bufs=2) as aux,
        tc.tile_pool(name="pq", bufs=1, space="PSUM") as pq,
        tc.tile_pool(name="pk", bufs=1, space="PSUM") as pk,
        tc.tile_pool(name="ppv", bufs=1, space="PSUM") as ppv,
        tc.tile_pool(name="pst", bufs=2, space="PSUM") as pst,
    ):
        # ---- resident constants / weights (all contiguous pre-tiled) ----
        # DMA order follows the critical path (HWDGE drains FIFO): the
        # weights the first k-tiles need, then block 0's first x tiles
        # interleaved with the rest of wq; the rope/mask/transpose consts
        # are issued inside block 0 right before their first consumers.
        wq_sb = wpool.tile([P, KT, HG], f32r, tag="wq")
        wk_sb = wpool.tile([P, KT, HEAD_DIM], f32r, tag="wk")
        wv_sb = wpool.tile([P, KT, HEAD_DIM], f32r, tag="wv")
        cos_sb = wpool.tile([P, S], f32, tag="cos")
        sinr_sb = wpool.tile([P, S], f32, tag="sinr")
        mask_sb = wpool.tile([P, 4, TQ], f32r, tag="mask")
        id_sb = wpool.tile([P, P], f32, tag="ident")
        idr_sb = wpool.tile([P, P], f32r, tag="identr")
        prot_sb = wpool.tile([P, P], f32r, tag="prot")
        ones_sb = wpool.tile([P, 1], f32r, tag="ones")
        # ones row for the PE partition-broadcast of 1/denominator
        # (keeps gpsimd free: a POOL-queue broadcast would serialize behind
        # the collective_compute wait and stall the next chunk's normalize)
        onesr_sb = wpool.tile([1, P], f32, tag="onesr")

        nc.sync.dma_start(wq_sb[:, 0:8, :], wqt[:, 0:8, :])
        nc.sync.dma_start(wk_sb[:], wkt)
        nc.sync.dma_start(wv_sb[:], wvt)
        x_pre = []
        for i in range(3):
            xtile = xpool.tile([P, KB, TQ], f32r, tag="x", name=f"xpre{i}")
            nc.sync.dma_start(xtile[:], xt[0, :, i * KB:(i + 1) * KB, :])
            x_pre.append(xtile)
            nc.sync.dma_start(wq_sb[:, (i + 1) * 8:(i + 2) * 8, :],
                              wqt[:, (i + 1) * 8:(i + 2) * 8, :])

        def _const_dmas():
            nc.sync.dma_start(cos_sb[:], cosT)
            nc.sync.dma_start(sinr_sb[:], sinrT)
            nc.sync.dma_start(id_sb[:], identIn)
            nc.sync.dma_start(idr_sb[:], identIn.bitcast(f32r))
            nc.sync.dma_start(prot_sb[:], protIn.bitcast(f32r))
            nc.sync.dma_start(ones_sb[:], onesIn)
            nc.sync.dma_start(onesr_sb[:], onesIn.bitcast(f32).rearrange(
                "p one -> one p"))
            nc.sync.dma_start(
                mask_sb[:],
                maskIn.bitcast(f32r).rearrange("p (r t) -> p r t", r=4))

        def rope(dst_f32r, src_ps, n):
            """dst = src*cos + rotate_half(src)*sin for token block n.

            src_ps is a [P, TQ] fp32 PSUM AP (projection output); the two
            DVE mults double as the PSUM evacuation.  The half-rotation
            runs on PE via the Prot permutation matmul."""
            cos_blk = cos_sb[:, n * TQ:(n + 1) * TQ]
            sinr_blk = sinr_sb[:, n * TQ:(n + 1) * TQ]
            qcos = rtmp.tile([P, TQ], f32, tag="qcos")
            nc.vector.tensor_tensor(qcos[:], src_ps, cos_blk, OP.mult)
            qsin = rtmp.tile([P, TQ], f32r, tag="qsin")
            nc.vector.tensor_tensor(qsin[:], src_ps, sinr_blk, OP.mult)
            rot_ps = pst.tile([P, TQ], f32, tag="st", name="rot")
            nc.tensor.matmul(rot_ps[:], prot_sb[:], qsin[:],
                             start=True, stop=True)
            nc.vector.tensor_tensor(dst_f32r, qcos[:], rot_ps[:], OP.add)

        for b in range(B):
            kT_cache = kvpool.tile([P, S], f32r, tag="kT")
            v_cache = kvpool.tile([P, S // P, HEAD_DIM], f32r, tag="v")
            for n in range(NB):
                blk = b * NB + n
                # ---------- QKV projection for this token block ----------
                q_ps = [
                    pq.tile([P, TQ], f32, tag=f"q{j}", name=f"qps{j}")
                    for j in range(QH)
                ]
                k_ps = pk.tile([P, TQ], f32, tag="kk", name="kps")
                v_ps = ppv.tile([P, TQ], f32, tag="pv", name="vps")
                for k8 in range(KT // KB):
                    if blk == 0 and k8 < 3:
                        x_t = x_pre[k8]
                    else:
                        x_t = xpool.tile([P, KB, TQ], f32r, tag="x")
                        nc.sync.dma_start(
                            x_t[:], xt[blk, :, k8 * KB:(k8 + 1) * KB, :])
                    for kk in range(KB):
                        k = k8 * KB + kk
                        st = dict(start=(k == 0), stop=(k == KT - 1))
                        for j in range(QH):
                            nc.tensor.matmul(
                                q_ps[j][:], wq_sb[:, k, j * P:(j + 1) * P],
                                x_t[:, kk, :], **st
                            )
                        nc.tensor.matmul(
                            k_ps[:], wk_sb[:, k, :], x_t[:, kk, :], **st)
                        nc.tensor.matmul(
                            v_ps[:], wv_sb[:, k, :], x_t[:, kk, :], **st)

                if blk == 0:
                    _const_dmas()
                # ---------- RoPE (also evacuates q/k PSUM banks) ----------
                qT_sb = qpool.tile([P, QH, TQ], f32r, tag="q")
                for j in range(QH):
                    rope(qT_sb[:, j, :], q_ps[j][:], n)
                rope(kT_cache[:, n * TQ:(n + 1) * TQ], k_ps[:], n)

                # ---------- V: evacuate + transpose to [tok, dim] ----------
                vT_sb = rtmp.tile([P, TQ], f32, tag="vtsb")
                nc.scalar.copy(vT_sb[:], v_ps[:])
                for j in range(4):
                    tp = pst.tile([P, TQ], f32, tag="st", name="vtp")
                    nc.tensor.transpose(
                        tp[:, :P], vT_sb[:, j * P:(j + 1) * P], id_sb[:]
                    )
                    nc.vector.tensor_copy(
                        v_cache[:, n * 4 + j, :], tp[:, :P]
                    )

                # ---------- attention, one GQA head at a time ----------
                ntk = (n + 1) * (TQ // P)
                for h in range(QH):
                    pv_ps = ppv.tile([P, TQ], f32, tag="pv", name="pvps")
                    dn_ps = pk.tile([P, TQ], f32, tag="kk", name="dnps")
                    qr = qT_sb[:, h, :]
                    for t in range(ntk):
                        diag = t >= ntk - 4
                        # diag position r: columns < 128r are fully masked
                        # (zero contribution) -> restrict every op to the
                        # live range [c0, TQ); bit-identical, less stream
                        r = t - (ntk - 4)
                        c0 = P * r if diag else 0
                        st_ps = pst.tile([P, TQ], f32, tag="st", name="stps")
                        nc.tensor.matmul(
                            st_ps[:, c0:], kT_cache[:, t * P:(t + 1) * P],
                            qr[:, c0:],
                            start=True, stop=not diag,
                        )
                        if diag:
                            # the triangular boundary lives in one strip
                            nc.tensor.matmul(
                                st_ps[:, c0:c0 + P], idr_sb[:],
                                mask_sb[:, r, c0:c0 + P],
                                start=False, stop=True,
                                skip_group_check=True,
                            )
                        es = epool.tile([P, TQ], f32r, tag="es")
                        nc.scalar.activation(es[:, c0:], st_ps[:, c0:],
                                             AF.Exp)
                        nc.tensor.matmul(
                            dn_ps[:1, c0:], ones_sb[:], es[:, c0:],
                            start=(t == 0), stop=(t == ntk - 1),
                            skip_group_check=True,
                        )
                        nc.tensor.matmul(
                            pv_ps[:, c0:], v_cache[:, t, :], es[:, c0:],
                            start=(t == 0), stop=(t == ntk - 1),
                            skip_group_check=True,
                        )
                    # normalize: 1/denominator broadcast over partitions
                    # (approx_fast: ~51 ULP, 5x faster than iterative divide;
                    # denominators are sums of exps, well inside safe range)
                    rec = aux.tile([1, TQ], f32, tag="rec")
                    nc.vector.reciprocal_approx_fast(rec[:], dn_ps[:1, :])
                    pv_sb = aux.tile([P, TQ], f32, tag="pvs")
                    nc.scalar.copy(pv_sb[:], pv_ps[:])
                    bc_ps = pst.tile([P, TQ], f32, tag="st", name="bc")
                    nc.tensor.matmul(bc_ps[:], onesr_sb[:], rec[:],
                                     start=True, stop=True)
                    ao = aux.tile([P, TQ], bf16, tag="ao")
                    nc.vector.tensor_tensor(ao[:], pv_sb[:], bc_ps[:], OP.mult)
                    ch = b * NB + n
                    nc.sync.dma_start(
                        ag_in[ch][h * P:(h + 1) * P, :], ao[:]
                    )

                # ---------- AllGather this chunk across the 8 cores ----------
                ch = b * NB + n
                if not getattr(nc, "_skip_collectives", False):
                    nc.gpsimd.collective_compute(
                        "AllGather",
                        mybir.AluOpType.bypass,
                        replica_groups=[list(range(NCORES))],
                        ins=[ag_in[ch].opt()],
                        outs=[ag_out[ch].opt()],
                    )

    # ---------- output projection: out[:, c*512:(c+1)*512] ----------
    KBO = 8  # k-tiles per at DMA (2MB chunks)
    NKG = KT // KBO
    with (
        tc.tile_pool(name="wopool", bufs=1) as wopool,
        tc.tile_pool(name="atpool", bufs=3) as atpool,
        tc.tile_pool(name="obpool", bufs=3) as obpool,
        tc.tile_pool(name="pop", bufs=1, space="PSUM") as pop,
    ):
        wo_sb = wopool.tile([P, KT, HG], bf16, tag="wo")
        nc.sync.dma_start(wo_sb[:, 0:8, :], wot[:, 0:8, :])
        for ch in range(NCHUNK):
            ag_r = ag_out[ch].rearrange("(ko p) t -> p ko t", p=P)
            # k-group pipeline: DMA of group kg+1 overlaps the 32 matmuls
            # consuming group kg; all 4 output banks accumulate per group.
            # wo chunks k8>=1 interleave with chunk 0's at loads so the
            # first matmuls start after ~4MB of DMA instead of ~10MB.
            op_ps = [
                pop.tile([P, TQ], f32, tag=f"op{m}", name=f"op{ch}_{m}")
                for m in range(HG // P)
            ]
            for kg in range(NKG):
                if ch == 0 and kg >= 1:
                    nc.sync.dma_start(wo_sb[:, kg * 8:(kg + 1) * 8, :],
                                      wot[:, kg * 8:(kg + 1) * 8, :])
                at = atpool.tile([P, KBO, TQ], bf16, tag="at",
                                 name=f"at{ch}_{kg}")
                nc.sync.dma_start(
                    at[:], ag_r[:, kg * KBO:(kg + 1) * KBO, :])
                for m in range(HG // P):
                    for kk in range(KBO):
                        nc.tensor.matmul(
                            op_ps[m][:], wo_sb[:, kg * KBO + kk,
                                               m * P:(m + 1) * P],
                            at[:, kk, :],
                            start=(kg == 0 and kk == 0),
                            stop=(kg == NKG - 1 and kk == KBO - 1),
                        )
            ob = obpool.tile([P, HG // P, TQ], f32, tag="ob")
            for m in range(HG // P):
                nc.vector.tensor_copy(ob[:, m, :], op_ps[m][:])
            nc.sync.dma_start(
                outT.rearrange("(m p) t -> p m t", p=P)
                    [:, :, ch * TQ:(ch + 1) * TQ], ob[:]
            )


_NC_CACHE = None


def _get_module():
    global _NC_CACHE
    if _NC_CACHE is None:
        _NC_CACHE = _build_module()
    return _NC_CACHE


def _host_consts():
    inv_freq = 1.0 / (ROPE_THETA ** (np.arange(0, HEAD_DIM, 2,
                                               dtype=np.float32) / HEAD_DIM))
    t = np.arange(S, dtype=np.float32)
    freqs = np.outer(t, inv_freq).astype(np.float32)      # [S, 64]
    cos_h = np.cos(freqs).T                               # [64, S]
    sin_h = np.sin(freqs).T
    cosT = np.concatenate([cos_h, cos_h], axis=0).astype(np.float32)
    # ssin = [-sin; sin];  sinrot[r] = ssin[(r+64)%128] = [sin; -sin]
    sinrT = np.concatenate([sin_h, -sin_h], axis=0).astype(np.float32)

    i = np.arange(P)[:, None]
    j = np.arange(TQ)[None, :]
    maskadd = np.concatenate(
        [np.where(i + r * P <= j, 0.0, MASKVAL).astype(np.float32)
         for r in range(4)], axis=1
    )                                                     # [128, 4*512]
    ident = np.eye(P, dtype=np.float32)
    prot = np.roll(np.eye(P, dtype=np.float32), 64, axis=0)
    ones = np.ones((P, 1), dtype=np.float32)
    return cosT, sinrT, maskadd, ident, prot, ones


def _tile_w(w):
    """[dims, HIDDEN] weight slice -> [P, KT, dims] pre-tiled layout."""
    return np.ascontiguousarray(
        w.T.reshape(KT, P, w.shape[0]).transpose(1, 0, 2))


def make_in_maps(hidden_states, wq, wk, wv, wo):
    hidden_states = np.asarray(hidden_states, dtype=np.float32)
    wq = np.asarray(wq, dtype=np.float32)
    wk = np.asarray(wk, dtype=np.float32)
    wv = np.asarray(wv, dtype=np.float32)
    wo = np.asarray(wo, dtype=np.float32)

    x2 = hidden_states.reshape(TOK, HIDDEN)
    # xt[blk, p, ko, t] = x2[blk*TQ + t, ko*P + p]
    xt = np.ascontiguousarray(
        x2.reshape(NBLK, TQ, KT, P).transpose(0, 3, 2, 1))
    cosT, sinrT, maskadd, ident, prot, ones = _host_consts()
    qscale = 1.0 / math.sqrt(HEAD_DIM)

    in_maps = []
    for c in range(NCORES):
        in_maps.append({
            "xt": xt,
            "wqt": _tile_w(wq[c * HG:(c + 1) * HG] * qscale),
            "wkt": _tile_w(wk[c * HEAD_DIM:(c + 1) * HEAD_DIM]),
            "wvt": _tile_w(wv[c * HEAD_DIM:(c + 1) * HEAD_DIM]),
            "wot": _tile_w(wo[c * HG:(c + 1) * HG]).astype(
                ml_dtypes.bfloat16),
            "cosT": cosT,
            "sinrT": sinrT,
            "maskIn": maskadd,
            "identIn": ident,
            "protIn": prot,
            "onesIn": ones,
        })
    return in_maps


def assemble_output(results):
    out = np.empty((TOK, HIDDEN), dtype=np.float32)
    for c in range(NCORES):
        out[:, c * HG:(c + 1) * HG] = results[c]["outT"].T
    return out.reshape(B, S, HIDDEN)


def kernel(hidden_states, wq, wk, wv, wo):
    nc = _get_module()
    in_maps = make_in_maps(hidden_states, wq, wk, wv, wo)
    trace = bool(int(os.environ.get("KERNEL_TRACE", "0")))
    res = bass_utils.run_bass_kernel_spmd(
        nc, in_maps, core_ids=list(range(NCORES)), trace=trace
    )
    if trace:
        kernel.last_results = res
    return assemble_output(res.results)


kernel.last_results = None



# revision 7
# speedup vs baseline: 13.2470x; 13.2470x over previous
"""InternLM3 attention block on 8 Trainium2 NeuronCores (Bass/Tile), v3.

Strategy (tensor-parallel over heads, collective-free):
  - 32 Q heads / 8 KV heads, head_dim 128.  Core c owns Q heads [4c,4c+4)
    and KV head c (one GQA group per core, so K/V never needs replication).
  - Per core, fused pipeline over 512-token blocks: QKV projection (fp32r
    matmuls, host-pretiled [p, ko, t] layouts so every DMA line is >=8KB
    contiguous) -> RoPE (2 DVE mults off PSUM + PE rotation matmul + DVE
    add) -> causal flash-style attention in S^T orientation:
      scores^T = kT-tile.T @ q^T  (PSUM), causal mask added via an
      identity-matmul of a -60000 bias tile (exp -> exact 0), exp on ACT,
      denominator accumulated on PE via a ones-column matmul, PV in PSUM,
      normalize with DVE reciprocal + a PE ones-row broadcast.
  - No collectives: instead of AllGather + column-sliced output projection,
    each core contracts its own 512 attention dims against its wo row-slice
    (wo[:, c*512:(c+1)*512]) producing a PARTIAL [4096, tok] output; the
    host sums the 8 partials.  Cores never synchronize, so per-core NEFF
    span is independent of launch skew and collective latency.
  - Host only shards/pre-tiles inputs and reduces the 8 partial outputs.

All matmuls run as float32r (TF32-like, full PE rate, ~1e-4 relative error);
the output projection runs bf16 weights x bf16 activations.
"""

import math
import os
import sys

if "/opt/trn_rl_repo" not in sys.path:
    sys.path.insert(0, "/opt/trn_rl_repo")

import ml_dtypes
import numpy as np

import concourse.bass as bass
import concourse.mybir as mybir
import concourse.tile as tile
from concourse import bacc
from concourse import bass_utils

# ---- problem constants (hardcoded per harness contract) ----
HIDDEN = 4096
N_HEADS = 32
N_KV_HEADS = 8
HEAD_DIM = 128
ROPE_THETA = 10000.0
B, S = 2, 2048
NCORES = 8

P = 128
TQ = 512                      # token block
NB = S // TQ                  # 4 blocks per batch
KT = HIDDEN // P              # 32 contraction tiles
QH = N_HEADS // NCORES        # 4 q-heads per core
HG = QH * HEAD_DIM            # 512 = head-group width per core
NBLK = B * NB                 # 8 token blocks
TOK = B * S                   # 4096 tokens
KB = 4                        # k-tiles per x DMA (1MB chunks)
MO = HIDDEN // P              # 32 output-dim tiles in the partial out-proj
MASKVAL = -60000.0            # exp(score + MASKVAL) == 0 exactly

f32 = mybir.dt.float32
f32r = mybir.dt.float32r
bf16 = mybir.dt.bfloat16


def _build_module():
    nc = bacc.Bacc("TRN2", target_bir_lowering=False, debug=False)

    xt = nc.dram_tensor("xt", [NBLK, P, KT, TQ], f32r,
                        kind="ExternalInput").ap()
    wqt = nc.dram_tensor("wqt", [P, KT, HG], f32r, kind="ExternalInput").ap()
    wkt = nc.dram_tensor("wkt", [P, KT, HEAD_DIM], f32r,
                         kind="ExternalInput").ap()
    wvt = nc.dram_tensor("wvt", [P, KT, HEAD_DIM], f32r,
                         kind="ExternalInput").ap()
    # wo row-slice for this core, pre-tiled: [p, k(4), HIDDEN]
    wot = nc.dram_tensor("wot", [P, HG // P, HIDDEN], bf16,
                         kind="ExternalInput").ap()
    cosT = nc.dram_tensor("cosT", [P, S], f32, kind="ExternalInput").ap()
    sinrT = nc.dram_tensor("sinrT", [P, S], f32, kind="ExternalInput").ap()
    maskIn = nc.dram_tensor("maskIn", [P, 4 * TQ], f32,
                            kind="ExternalInput").ap()
    identIn = nc.dram_tensor("identIn", [P, P], f32, kind="ExternalInput").ap()
    protIn = nc.dram_tensor("protIn", [P, P], f32, kind="ExternalInput").ap()
    onesIn = nc.dram_tensor("onesIn", [P, 1], f32r, kind="ExternalInput").ap()
    # partial output projection: out[p, m, t] = out-dim (m*128+p), token t
    outT = nc.dram_tensor("outT", [P, MO, TOK], bf16,
                          kind="ExternalOutput").ap()

    ao_dram = [
        nc.dram_tensor(f"ao{i}", [HG, TQ], bf16, kind="Internal").ap()
        for i in range(NBLK)
    ]

    with tile.TileContext(nc) as tc:
        _body(tc, nc, xt, wqt, wkt, wvt, wot, cosT, sinrT, maskIn, identIn,
              protIn, onesIn, outT, ao_dram)
    nc.compile()
    return nc


def _body(tc, nc, xt, wqt, wkt, wvt, wot, cosT, sinrT, maskIn, identIn,
          protIn, onesIn, outT, ao_dram):
    AF = mybir.ActivationFunctionType
    OP = mybir.AluOpType

    with (
        tc.tile_pool(name="wpool", bufs=1) as wpool,
        tc.tile_pool(name="xpool", bufs=3) as xpool,
        tc.tile_pool(name="kvpool", bufs=1) as kvpool,
        tc.tile_pool(name="qpool", bufs=1) as qpool,
        tc.tile_pool(name="rtmp", bufs=2) as rtmp,
        tc.tile_pool(name="epool", bufs=4) as epool,
        tc.tile_pool(name="aux", bufs=2) as aux,
        tc.tile_pool(name="pq", bufs=1, space="PSUM") as pq,
        tc.tile_pool(name="pk", bufs=1, space="PSUM") as pk,
        tc.tile_pool(name="ppv", bufs=1, space="PSUM") as ppv,
        tc.tile_pool(name="pst", bufs=2, space="PSUM") as pst,
    ):
        # ---- resident constants / weights (all contiguous pre-tiled) ----
        # DMA order follows the critical path (HWDGE drains FIFO): the
        # weights the first k-tiles need, then block 0's first x tiles
        # interleaved with the rest of wq; the rope/mask/transpose consts
        # are issued inside block 0 right before their first consumers.
        wq_sb = wpool.tile([P, KT, HG], f32r, tag="wq")
        wk_sb = wpool.tile([P, KT, HEAD_DIM], f32r, tag="wk")
        wv_sb = wpool.tile([P, KT, HEAD_DIM], f32r, tag="wv")
        cos_sb = wpool.tile([P, S], f32, tag="cos")
        sinr_sb = wpool.tile([P, S], f32, tag="sinr")
        mask_sb = wpool.tile([P, 4, TQ], f32r, tag="mask")
        id_sb = wpool.tile([P, P], f32, tag="ident")
        idr_sb = wpool.tile([P, P], f32r, tag="identr")
        prot_sb = wpool.tile([P, P], f32r, tag="prot")
        ones_sb = wpool.tile([P, 1], f32r, tag="ones")
        # ones row for the PE partition-broadcast of 1/denominator
        onesr_sb = wpool.tile([1, P], f32, tag="onesr")

        nc.sync.dma_start(wq_sb[:, 0:8, :], wqt[:, 0:8, :])
        nc.sync.dma_start(wk_sb[:], wkt)
        nc.sync.dma_start(wv_sb[:], wvt)
        x_pre = []
        for i in range(3):
            xtile = xpool.tile([P, KB, TQ], f32r, tag="x", name=f"xpre{i}")
            nc.sync.dma_start(xtile[:], xt[0, :, i * KB:(i + 1) * KB, :])
            x_pre.append(xtile)
            nc.sync.dma_start(wq_sb[:, (i + 1) * 8:(i + 2) * 8, :],
                              wqt[:, (i + 1) * 8:(i + 2) * 8, :])

        def _const_dmas():
            nc.sync.dma_start(cos_sb[:], cosT)
            nc.sync.dma_start(sinr_sb[:], sinrT)
            nc.sync.dma_start(id_sb[:], identIn)
            nc.sync.dma_start(idr_sb[:], identIn.bitcast(f32r))
            nc.sync.dma_start(prot_sb[:], protIn.bitcast(f32r))
            nc.sync.dma_start(ones_sb[:], onesIn)
            nc.sync.dma_start(onesr_sb[:], onesIn.bitcast(f32).rearrange(
                "p one -> one p"))
            nc.sync.dma_start(
                mask_sb[:],
                maskIn.bitcast(f32r).rearrange("p (r t) -> p r t", r=4))

        def rope(dst_f32r, src_ps, n):
            """dst = src*cos + rotate_half(src)*sin for token block n.

            src_ps is a [P, TQ] fp32 PSUM AP (projection output); the two
            DVE mults double as the PSUM evacuation.  The half-rotation
            runs on PE via the Prot permutation matmul."""
            cos_blk = cos_sb[:, n * TQ:(n + 1) * TQ]
            sinr_blk = sinr_sb[:, n * TQ:(n + 1) * TQ]
            qcos = rtmp.tile([P, TQ], f32, tag="qcos")
            nc.vector.tensor_tensor(qcos[:], src_ps, cos_blk, OP.mult)
            qsin = rtmp.tile([P, TQ], f32r, tag="qsin")
            nc.vector.tensor_tensor(qsin[:], src_ps, sinr_blk, OP.mult)
            rot_ps = pst.tile([P, TQ], f32, tag="st", name="rot")
            nc.tensor.matmul(rot_ps[:], prot_sb[:], qsin[:],
                             start=True, stop=True)
            nc.vector.tensor_tensor(dst_f32r, qcos[:], rot_ps[:], OP.add)

        for b in range(B):
            kT_cache = kvpool.tile([P, S], f32r, tag="kT")
            v_cache = kvpool.tile([P, S // P, HEAD_DIM], f32r, tag="v")
            for n in range(NB):
                blk = b * NB + n
                # ---------- QKV projection for this token block ----------
                q_ps = [
                    pq.tile([P, TQ], f32, tag=f"q{j}", name=f"qps{j}")
                    for j in range(QH)
                ]
                k_ps = pk.tile([P, TQ], f32, tag="kk", name="kps")
                v_ps = ppv.tile([P, TQ], f32, tag="pv", name="vps")
                for k8 in range(KT // KB):
                    if blk == 0 and k8 < 3:
                        x_t = x_pre[k8]
                    else:
                        x_t = xpool.tile([P, KB, TQ], f32r, tag="x")
                        nc.sync.dma_start(
                            x_t[:], xt[blk, :, k8 * KB:(k8 + 1) * KB, :])
                    for kk in range(KB):
                        k = k8 * KB + kk
                        st = dict(start=(k == 0), stop=(k == KT - 1))
                        for j in range(QH):
                            nc.tensor.matmul(
                                q_ps[j][:], wq_sb[:, k, j * P:(j + 1) * P],
                                x_t[:, kk, :], **st
                            )
                        nc.tensor.matmul(
                            k_ps[:], wk_sb[:, k, :], x_t[:, kk, :], **st)
                        nc.tensor.matmul(
                            v_ps[:], wv_sb[:, k, :], x_t[:, kk, :], **st)

                if blk == 0:
                    _const_dmas()
                # ---------- RoPE (also evacuates q/k PSUM banks) ----------
                qT_sb = qpool.tile([P, QH, TQ], f32r, tag="q")
                for j in range(QH):
                    rope(qT_sb[:, j, :], q_ps[j][:], n)
                rope(kT_cache[:, n * TQ:(n + 1) * TQ], k_ps[:], n)

                # ---------- V: evacuate + transpose to [tok, dim] ----------
                vT_sb = rtmp.tile([P, TQ], f32, tag="vtsb")
                nc.scalar.copy(vT_sb[:], v_ps[:])
                for j in range(4):
                    tp = pst.tile([P, TQ], f32, tag="st", name="vtp")
                    nc.tensor.transpose(
                        tp[:, :P], vT_sb[:, j * P:(j + 1) * P], id_sb[:]
                    )
                    nc.vector.tensor_copy(
                        v_cache[:, n * 4 + j, :], tp[:, :P]
                    )

                # ---------- attention, one GQA head at a time ----------
                ntk = (n + 1) * (TQ // P)
                for h in range(QH):
                    pv_ps = ppv.tile([P, TQ], f32, tag="pv", name="pvps")
                    dn_ps = pk.tile([P, TQ], f32, tag="kk", name="dnps")
                    qr = qT_sb[:, h, :]
                    for t in range(ntk):
                        diag = t >= ntk - 4
                        # diag position r: columns < 128r are fully masked
                        # (zero contribution) -> restrict every op to the
                        # live range [c0, TQ); bit-identical, less stream
                        r = t - (ntk - 4)
                        c0 = P * r if diag else 0
                        st_ps = pst.tile([P, TQ], f32, tag="st", name="stps")
                        nc.tensor.matmul(
                            st_ps[:, c0:], kT_cache[:, t * P:(t + 1) * P],
                            qr[:, c0:],
                            start=True, stop=not diag,
                        )
                        if diag:
                            # the triangular boundary lives in one strip
                            nc.tensor.matmul(
                                st_ps[:, c0:c0 + P], idr_sb[:],
                                mask_sb[:, r, c0:c0 + P],
                                start=False, stop=True,
                                skip_group_check=True,
                            )
                        es = epool.tile([P, TQ], f32r, tag="es")
                        nc.scalar.activation(es[:, c0:], st_ps[:, c0:],
                                             AF.Exp)
                        nc.tensor.matmul(
                            dn_ps[:1, c0:], ones_sb[:], es[:, c0:],
                            start=(t == 0), stop=(t == ntk - 1),
                            skip_group_check=True,
                        )
                        nc.tensor.matmul(
                            pv_ps[:, c0:], v_cache[:, t, :], es[:, c0:],
                            start=(t == 0), stop=(t == ntk - 1),
                            skip_group_check=True,
                        )
                    # normalize: 1/denominator broadcast over partitions
                    # (approx_fast: ~51 ULP, 5x faster than iterative divide;
                    # denominators are sums of exps, well inside safe range)
                    rec = aux.tile([1, TQ], f32, tag="rec")
                    nc.vector.reciprocal_approx_fast(rec[:], dn_ps[:1, :])
                    pv_sb = aux.tile([P, TQ], f32, tag="pvs")
                    nc.scalar.copy(pv_sb[:], pv_ps[:])
                    bc_ps = pst.tile([P, TQ], f32, tag="st", name="bc")
                    nc.tensor.matmul(bc_ps[:], onesr_sb[:], rec[:],
                                     start=True, stop=True)
                    ao = aux.tile([P, TQ], bf16, tag="ao")
                    nc.vector.tensor_tensor(ao[:], pv_sb[:], bc_ps[:], OP.mult)
                    nc.sync.dma_start(
                        ao_dram[blk][h * P:(h + 1) * P, :], ao[:]
                    )

    # ---------- phase 2: partial output projection (no collective) ----
    # out[m*128+p, t] = sum_k wo[m*128+p, cHG + k*128+j] * ao[k*128+j, t]
    # contraction over this core's 512 attention dims only; the host sums
    # the 8 per-core partials.
    with (
        tc.tile_pool(name="wopool", bufs=1) as wopool,
        tc.tile_pool(name="atpool", bufs=3) as atpool,
        tc.tile_pool(name="obpool", bufs=3) as obpool,
        tc.tile_pool(name="pop", bufs=2, space="PSUM") as pop,
    ):
        wo_sb = wopool.tile([P, HG // P, HIDDEN], bf16, tag="wo")
        nc.sync.dma_start(wo_sb[:], wot)
        for ch in range(NBLK):
            ao_r = ao_dram[ch].rearrange("(k p) t -> p k t", p=P)
            at = atpool.tile([P, HG // P, TQ], bf16, tag="at",
                             name=f"at{ch}")
            nc.sync.dma_start(at[:], ao_r)
            for m2 in range(MO // 4):
                op_ps = [
                    pop.tile([P, TQ], f32, tag=f"op{j}",
                             name=f"op{ch}_{m2}_{j}")
                    for j in range(4)
                ]
                ob = obpool.tile([P, 4, TQ], bf16, tag="ob")
                for j in range(4):
                    m = m2 * 4 + j
                    for k in range(HG // P):
                        nc.tensor.matmul(
                            op_ps[j][:],
                            wo_sb[:, k, m * P:(m + 1) * P],
                            at[:, k, :],
                            start=(k == 0), stop=(k == HG // P - 1),
                        )
                    nc.vector.tensor_copy(ob[:, j, :], op_ps[j][:])
                nc.sync.dma_start(
                    outT[:, m2 * 4:(m2 + 1) * 4, ch * TQ:(ch + 1) * TQ],
                    ob[:]
                )


_NC_CACHE = None


def _get_module():
    global _NC_CACHE
    if _NC_CACHE is None:
        _NC_CACHE = _build_module()
    return _NC_CACHE


def _host_consts():
    inv_freq = 1.0 / (ROPE_THETA ** (np.arange(0, HEAD_DIM, 2,
                                               dtype=np.float32) / HEAD_DIM))
    t = np.arange(S, dtype=np.float32)
    freqs = np.outer(t, inv_freq).astype(np.float32)      # [S, 64]
    cos_h = np.cos(freqs).T                               # [64, S]
    sin_h = np.sin(freqs).T
    cosT = np.concatenate([cos_h, cos_h], axis=0).astype(np.float32)
    # ssin = [-sin; sin];  sinrot[r] = ssin[(r+64)%128] = [sin; -sin]
    sinrT = np.concatenate([sin_h, -sin_h], axis=0).astype(np.float32)

    i = np.arange(P)[:, None]
    j = np.arange(TQ)[None, :]
    maskadd = np.concatenate(
        [np.where(i + r * P <= j, 0.0, MASKVAL).astype(np.float32)
         for r in range(4)], axis=1
    )                                                     # [128, 4*512]
    ident = np.eye(P, dtype=np.float32)
    prot = np.roll(np.eye(P, dtype=np.float32), 64, axis=0)
    ones = np.ones((P, 1), dtype=np.float32)
    return cosT, sinrT, maskadd, ident, prot, ones


def _tile_w(w):
    """[dims, HIDDEN] weight slice -> [P, KT, dims] pre-tiled layout."""
    return np.ascontiguousarray(
        w.T.reshape(KT, P, w.shape[0]).transpose(1, 0, 2))


def make_in_maps(hidden_states, wq, wk, wv, wo):
    hidden_states = np.asarray(hidden_states, dtype=np.float32)
    wq = np.asarray(wq, dtype=np.float32)
    wk = np.asarray(wk, dtype=np.float32)
    wv = np.asarray(wv, dtype=np.float32)
    wo = np.asarray(wo, dtype=np.float32)

    x2 = hidden_states.reshape(TOK, HIDDEN)
    # xt[blk, p, ko, t] = x2[blk*TQ + t, ko*P + p]
    xt = np.ascontiguousarray(
        x2.reshape(NBLK, TQ, KT, P).transpose(0, 3, 2, 1))
    cosT, sinrT, maskadd, ident, prot, ones = _host_consts()
    qscale = 1.0 / math.sqrt(HEAD_DIM)

    in_maps = []
    for c in range(NCORES):
        # wo row-slice [HIDDEN, HG] -> pre-tiled [P, HG//P, HIDDEN]:
        # wot[p, k, d] = wo[d, c*HG + k*128 + p]
        wo_sl = wo[:, c * HG:(c + 1) * HG]                # [HIDDEN, HG]
        wot = np.ascontiguousarray(
            wo_sl.T.reshape(HG // P, P, HIDDEN).transpose(1, 0, 2)
        ).astype(ml_dtypes.bfloat16)
        in_maps.append({
            "xt": xt,
            "wqt": _tile_w(wq[c * HG:(c + 1) * HG] * qscale),
            "wkt": _tile_w(wk[c * HEAD_DIM:(c + 1) * HEAD_DIM]),
            "wvt": _tile_w(wv[c * HEAD_DIM:(c + 1) * HEAD_DIM]),
            "wot": wot,
            "cosT": cosT,
            "sinrT": sinrT,
            "maskIn": maskadd,
            "identIn": ident,
            "protIn": prot,
            "onesIn": ones,
        })
    return in_maps


def assemble_output(results):
    # outT per core: [P, MO, TOK] bf16 partials; out[d, t] = sum_c
    # part_c[d % 128, d // 128, t]
    acc = np.zeros((P, MO, TOK), dtype=np.float32)
    for c in range(NCORES):
        acc += results[c]["outT"].astype(np.float32)
    out = acc.transpose(1, 0, 2).reshape(HIDDEN, TOK)
    return np.ascontiguousarray(out.T).reshape(B, S, HIDDEN)


def kernel(hidden_states, wq, wk, wv, wo):
    nc = _get_module()
    in_maps = make_in_maps(hidden_states, wq, wk, wv, wo)
    trace = bool(int(os.environ.get("KERNEL_TRACE", "0")))
    res = bass_utils.run_bass_kernel_spmd(
        nc, in_maps, core_ids=list(range(NCORES)), trace=trace
    )
    if trace:
        kernel.last_results = res
    return assemble_output(res.results)


kernel.last_results = None


# revision 17
# speedup vs baseline: 24.2923x; 1.8338x over previous
"""InternLM3 attention block on 8 Trainium2 NeuronCores (Bass/Tile), v4.

Strategy (tensor-parallel over heads, collective-free):
  - 32 Q heads / 8 KV heads, head_dim 128.  Core c owns Q heads [4c,4c+4)
    and KV head c (one GQA group per core, so K/V never needs replication).
  - Per core, fused pipeline over 512-token blocks: QKV projection (fp32r
    matmuls, host-pretiled packed [p, ko, 768] layout so every DMA line is
    large and contiguous) -> RoPE (2 DVE mults off PSUM + PE rotation
    matmul + DVE add) -> causal flash-style attention in S^T orientation:
      scores^T = kT-tile.T @ q^T  (PSUM, fp32r width-floored at 256 since
      narrow fp32r runs at 1/4 PE rate), causal mask added via a bf16
      identity-matmul of a -60000 bias strip (exp -> exact 0), exp on ACT
      (bf16 out), denominator accumulated on PE via a bf16 ones-column
      matmul, PV in PSUM (bf16 operands, full rate at any width),
      normalize with DVE reciprocal + GpSimd partition_broadcast.
  - No collectives: instead of AllGather + column-sliced output projection,
    each core contracts its own 512 attention dims against its wo row-slice
    (wo[:, c*512:(c+1)*512]) producing a PARTIAL [4096, tok] output; the
    host sums the 8 partials.  Cores never synchronize, so per-core NEFF
    span is independent of launch skew and collective latency.
  - Inputs are packed into 4 device tensors (xt, wqkv, wot, consts) —
    per-argument dispatch overhead through the PJRT path is significant.
  - DMA is split across both HWDGE rings (sync: loads, scalar: stores).

Projection matmuls run fp32r (TF32-like, full PE rate at >=256 wide,
~1e-4 relative error); attention probabilities and the output projection
run bf16 (measured end-to-end error ~3e-3 against the fp32 reference).
"""

import math
import os
import sys

if "/opt/trn_rl_repo" not in sys.path:
    sys.path.insert(0, "/opt/trn_rl_repo")

import ml_dtypes
import numpy as np

import concourse.bass as bass
import concourse.mybir as mybir
import concourse.tile as tile
from concourse import bacc
from concourse import bass_utils

# ---- problem constants (hardcoded per harness contract) ----
HIDDEN = 4096
N_HEADS = 32
N_KV_HEADS = 8
HEAD_DIM = 128
ROPE_THETA = 10000.0
B, S = 2, 2048
NCORES = 8

P = 128
TQ = 512                      # token block
NB = S // TQ                  # 4 blocks per batch
KT = HIDDEN // P              # 32 contraction tiles
QH = N_HEADS // NCORES        # 4 q-heads per core
HG = QH * HEAD_DIM            # 512 = head-group width per core
WKV = HG + 2 * HEAD_DIM       # 768 = packed wq|wk|wv width
NBLK = B * NB                 # 8 token blocks
TOK = B * S                   # 4096 tokens
KB = 4                        # k-tiles per x DMA (1.5MB wqkv chunks)
MO = HIDDEN // P              # 32 output-dim tiles in the partial out-proj
MASKVAL = -60000.0            # exp(score + MASKVAL) == 0 exactly

# packed consts layout (f32 columns)
C_COS = 0
C_SINR = C_COS + S
C_MASK = C_SINR + S
C_ID = C_MASK + 4 * TQ
C_PROT = C_ID + P
C_ONES = C_PROT + P
C_COLS = C_ONES + 1

f32 = mybir.dt.float32
f32r = mybir.dt.float32r
bf16 = mybir.dt.bfloat16


def _build_module():
    nc = bacc.Bacc("TRN2", target_bir_lowering=False, debug=False)

    xt = nc.dram_tensor("xt", [NBLK, P, KT, TQ], f32r,
                        kind="ExternalInput").ap()
    wqkv = nc.dram_tensor("wqkv", [P, KT, WKV], f32r,
                          kind="ExternalInput").ap()
    # wo row-slice for this core, pre-tiled: [p, k(4), HIDDEN]
    wot = nc.dram_tensor("wot", [P, HG // P, HIDDEN], bf16,
                         kind="ExternalInput").ap()
    consts = nc.dram_tensor("consts", [P, C_COLS], f32,
                            kind="ExternalInput").ap()
    # partial output projection: out[p, m, t] = out-dim (m*128+p), token t
    outT = nc.dram_tensor("outT", [P, MO, TOK], bf16,
                          kind="ExternalOutput").ap()

    ao_dram = [
        nc.dram_tensor(f"ao{i}", [HG, TQ], bf16, kind="Internal").ap()
        for i in range(NBLK)
    ]

    with tile.TileContext(nc) as tc:
        _body(tc, nc, xt, wqkv, wot, consts, outT, ao_dram)
    nc.compile()
    return nc


def _body(tc, nc, xt, wqkv, wot, consts, outT, ao_dram):
    AF = mybir.ActivationFunctionType
    OP = mybir.AluOpType

    with (
        tc.tile_pool(name="wpool", bufs=1) as wpool,
        tc.tile_pool(name="xpool", bufs=3) as xpool,
        tc.tile_pool(name="kvpool", bufs=1) as kvpool,
        tc.tile_pool(name="qpool", bufs=1) as qpool,
        tc.tile_pool(name="rtmp", bufs=2) as rtmp,
        tc.tile_pool(name="epool", bufs=4) as epool,
        tc.tile_pool(name="aux", bufs=2) as aux,
        tc.tile_pool(name="pq", bufs=1, space="PSUM") as pq,
        tc.tile_pool(name="pk", bufs=1, space="PSUM") as pk,
        tc.tile_pool(name="ppv", bufs=1, space="PSUM") as ppv,
        tc.tile_pool(name="pst", bufs=2, space="PSUM") as pst,
    ):
        # ---- resident weights / constants (packed, pre-tiled) ----
        # DMA order follows the critical path (HWDGE drains FIFO): the
        # k-tiles the first matmuls need, then block 0's x tiles
        # interleaved with the rest of wqkv; the packed consts are issued
        # inside block 0 right before their first consumers.
        wqkv_sb = wpool.tile([P, KT, WKV], f32r, tag="wqkv")
        c_sb = wpool.tile([P, C_COLS], f32, tag="consts")
        mask_sb = wpool.tile([P, 4 * TQ], bf16, tag="mask")
        idb_sb = wpool.tile([P, P], bf16, tag="identb")
        ones_sb = wpool.tile([P, 1], bf16, tag="ones")
        # fp32r matmul operands must be PRODUCED as fp32r (BIR verifier),
        # so prot gets its own tile, DMA'd with the bitcast on the DRAM side
        prot_sb = wpool.tile([P, P], f32r, tag="prot")

        cos_sb = c_sb[:, C_COS:C_COS + S]
        sinr_sb = c_sb[:, C_SINR:C_SINR + S]
        id_sb = c_sb[:, C_ID:C_ID + P]

        nc.sync.dma_start(wqkv_sb[:, 0:8, :], wqkv[:, 0:8, :])
        x_pre = []
        for i in range(3):
            xtile = xpool.tile([P, KB, TQ], f32r, tag="x", name=f"xpre{i}")
            nc.sync.dma_start(xtile[:], xt[0, :, i * KB:(i + 1) * KB, :])
            x_pre.append(xtile)
            nc.sync.dma_start(wqkv_sb[:, (i + 1) * 8:(i + 2) * 8, :],
                              wqkv[:, (i + 1) * 8:(i + 2) * 8, :])

        def _const_setup():
            nc.sync.dma_start(c_sb[:], consts)
            nc.sync.dma_start(
                prot_sb[:], consts.bitcast(f32r)[:, C_PROT:C_PROT + P])
            # bf16 working copies (bf16 matmul operands run 1 cycle/row on
            # PE at any width; narrow fp32r would run at 1/4 rate)
            nc.vector.tensor_copy(mask_sb[:],
                                  c_sb[:, C_MASK:C_MASK + 4 * TQ])
            nc.vector.tensor_copy(idb_sb[:], id_sb)
            nc.vector.tensor_copy(ones_sb[:], c_sb[:, C_ONES:C_ONES + 1])

        def rope(dst_f32r, src_ps, n):
            """dst = src*cos + rotate_half(src)*sin for token block n.

            src_ps is a [P, TQ] fp32 PSUM AP (projection output); the two
            DVE mults double as the PSUM evacuation.  The half-rotation
            runs on PE via the Prot permutation matmul."""
            cos_blk = cos_sb[:, n * TQ:(n + 1) * TQ]
            sinr_blk = sinr_sb[:, n * TQ:(n + 1) * TQ]
            qcos = rtmp.tile([P, TQ], f32, tag="qcos")
            nc.vector.tensor_tensor(qcos[:], src_ps, cos_blk, OP.mult)
            qsin = rtmp.tile([P, TQ], f32r, tag="qsin")
            nc.vector.tensor_tensor(qsin[:], src_ps, sinr_blk, OP.mult)
            rot_ps = pst.tile([P, TQ], f32, tag="st", name="rot")
            nc.tensor.matmul(rot_ps[:], prot_sb[:], qsin[:],
                             start=True, stop=True)
            nc.vector.tensor_tensor(dst_f32r, qcos[:], rot_ps[:], OP.add)

        for b in range(B):
            kT_cache = kvpool.tile([P, S], f32r, tag="kT")
            v_cache = kvpool.tile([P, S // P, HEAD_DIM], bf16, tag="v")
            for n in range(NB):
                blk = b * NB + n
                # ---------- QKV projection for this token block ----------
                q_ps = [
                    pq.tile([P, TQ], f32, tag=f"q{j}", name=f"qps{j}")
                    for j in range(QH)
                ]
                k_ps = pk.tile([P, TQ], f32, tag="kk", name="kps")
                v_ps = ppv.tile([P, TQ], f32, tag="pv", name="vps")
                for k8 in range(KT // KB):
                    if blk == 0 and k8 < 3:
                        x_t = x_pre[k8]
                    else:
                        x_t = xpool.tile([P, KB, TQ], f32r, tag="x")
                        nc.sync.dma_start(
                            x_t[:], xt[blk, :, k8 * KB:(k8 + 1) * KB, :])
                    for kk in range(KB):
                        k = k8 * KB + kk
                        st = dict(start=(k == 0), stop=(k == KT - 1))
                        for j in range(QH):
                            nc.tensor.matmul(
                                q_ps[j][:],
                                wqkv_sb[:, k, j * P:(j + 1) * P],
                                x_t[:, kk, :], **st
                            )
                        nc.tensor.matmul(
                            k_ps[:], wqkv_sb[:, k, HG:HG + HEAD_DIM],
                            x_t[:, kk, :], **st)
                        nc.tensor.matmul(
                            v_ps[:],
                            wqkv_sb[:, k, HG + HEAD_DIM:HG + 2 * HEAD_DIM],
                            x_t[:, kk, :], **st)

                if blk == 0:
                    _const_setup()
                # ---------- RoPE (also evacuates q/k PSUM banks) ----------
                qT_sb = qpool.tile([P, QH, TQ], f32r, tag="q")
                for j in range(QH):
                    rope(qT_sb[:, j, :], q_ps[j][:], n)
                rope(kT_cache[:, n * TQ:(n + 1) * TQ], k_ps[:], n)

                # ---------- V: evacuate + transpose to [tok, dim] ----------
                vT_sb = rtmp.tile([P, TQ], f32, tag="vtsb")
                nc.scalar.copy(vT_sb[:], v_ps[:])
                for j in range(4):
                    tp = pst.tile([P, TQ], f32, tag="st", name="vtp")
                    nc.tensor.transpose(
                        tp[:, :P], vT_sb[:, j * P:(j + 1) * P], id_sb
                    )
                    nc.vector.tensor_copy(
                        v_cache[:, n * 4 + j, :], tp[:, :P]
                    )

                # ---------- attention, one GQA head at a time ----------
                ntk = (n + 1) * (TQ // P)
                for h in range(QH):
                    pv_ps = ppv.tile([P, TQ], f32, tag="pv", name="pvps")
                    dn_ps = pk.tile([P, TQ], f32, tag="kk", name="dnps")
                    qr = qT_sb[:, h, :]
                    for t in range(ntk):
                        diag = t >= ntk - 4
                        # diag position r: columns < 128r are fully masked
                        # (zero contribution) -> restrict every op to the
                        # live range [c0, TQ); bit-identical, less stream.
                        # The f32r score matmul floors its width at 256
                        # (f32r below 256 wide runs at 1/4 PE rate, so 256
                        # columns are cheaper than 128); extra columns land
                        # in PSUM but are never read.
                        r = t - (ntk - 4)
                        c0 = P * r if diag else 0
                        lo = min(c0, TQ - 256) if diag else 0
                        st_ps = pst.tile([P, TQ], f32, tag="st", name="stps")
                        nc.tensor.matmul(
                            st_ps[:, lo:], kT_cache[:, t * P:(t + 1) * P],
                            qr[:, lo:],
                            start=True, stop=not diag,
                        )
                        if diag:
                            # the triangular boundary lives in one strip
                            nc.tensor.matmul(
                                st_ps[:, c0:c0 + P], idb_sb[:],
                                mask_sb[:, r * TQ + c0:r * TQ + c0 + P],
                                start=False, stop=True,
                                skip_group_check=True,
                            )
                        es = epool.tile([P, TQ], bf16, tag="es")
                        nc.scalar.activation(es[:, c0:], st_ps[:, c0:],
                                             AF.Exp)
                        nc.tensor.matmul(
                            dn_ps[:1, c0:], ones_sb[:], es[:, c0:],
                            start=(t == 0), stop=(t == ntk - 1),
                            skip_group_check=True,
                        )
                        nc.tensor.matmul(
                            pv_ps[:, c0:], v_cache[:, t, :], es[:, c0:],
                            start=(t == 0), stop=(t == ntk - 1),
                            skip_group_check=True,
                        )
                    # normalize: 1/denominator broadcast over partitions
                    # (approx_fast: ~51 ULP, 5x faster than iterative divide;
                    # denominators are sums of exps, well inside safe range).
                    # The broadcast runs on the otherwise-idle GpSimd engine
                    # (no collectives in this kernel to serialize behind).
                    rec = aux.tile([1, TQ], f32, tag="rec")
                    nc.vector.reciprocal_approx_fast(rec[:], dn_ps[:1, :])
                    pv_sb = aux.tile([P, TQ], f32, tag="pvs")
                    nc.scalar.copy(pv_sb[:], pv_ps[:])
                    bcb = aux.tile([P, TQ], f32, tag="bcb")
                    nc.gpsimd.partition_broadcast(bcb[:], rec[:1, :],
                                                  channels=P)
                    ao = aux.tile([P, TQ], bf16, tag="ao")
                    nc.vector.tensor_tensor(ao[:], pv_sb[:], bcb[:], OP.mult)
                    nc.scalar.dma_start(
                        ao_dram[blk][h * P:(h + 1) * P, :], ao[:]
                    )

    # ---------- phase 2: partial output projection (no collective) ----
    # out[m*128+p, t] = sum_k wo[m*128+p, cHG + k*128+j] * ao[k*128+j, t]
    # contraction over this core's 512 attention dims only; the host sums
    # the 8 per-core partials.
    with (
        tc.tile_pool(name="wopool", bufs=1) as wopool,
        tc.tile_pool(name="atpool", bufs=3) as atpool,
        tc.tile_pool(name="obpool", bufs=3) as obpool,
        tc.tile_pool(name="pop", bufs=2, space="PSUM") as pop,
    ):
        wo_sb = wopool.tile([P, HG // P, HIDDEN], bf16, tag="wo")
        nc.sync.dma_start(wo_sb[:], wot)
        for ch in range(NBLK):
            ao_r = ao_dram[ch].rearrange("(k p) t -> p k t", p=P)
            at = atpool.tile([P, HG // P, TQ], bf16, tag="at",
                             name=f"at{ch}")
            nc.sync.dma_start(at[:], ao_r)
            for m2 in range(MO // 4):
                op_ps = [
                    pop.tile([P, TQ], f32, tag=f"op{j}",
                             name=f"op{ch}_{m2}_{j}")
                    for j in range(4)
                ]
                ob = obpool.tile([P, 4, TQ], bf16, tag="ob")
                for j in range(4):
                    m = m2 * 4 + j
                    for k in range(HG // P):
                        nc.tensor.matmul(
                            op_ps[j][:],
                            wo_sb[:, k, m * P:(m + 1) * P],
                            at[:, k, :],
                            start=(k == 0), stop=(k == HG // P - 1),
                        )
                    nc.vector.tensor_copy(ob[:, j, :], op_ps[j][:])
                nc.scalar.dma_start(
                    outT[:, m2 * 4:(m2 + 1) * 4, ch * TQ:(ch + 1) * TQ],
                    ob[:]
                )


_NC_CACHE = None


def _get_module():
    global _NC_CACHE
    if _NC_CACHE is None:
        _NC_CACHE = _build_module()
    return _NC_CACHE


def _host_consts():
    inv_freq = 1.0 / (ROPE_THETA ** (np.arange(0, HEAD_DIM, 2,
                                               dtype=np.float32) / HEAD_DIM))
    t = np.arange(S, dtype=np.float32)
    freqs = np.outer(t, inv_freq).astype(np.float32)      # [S, 64]
    cos_h = np.cos(freqs).T                               # [64, S]
    sin_h = np.sin(freqs).T
    cosT = np.concatenate([cos_h, cos_h], axis=0).astype(np.float32)
    # ssin = [-sin; sin];  sinrot[r] = ssin[(r+64)%128] = [sin; -sin]
    sinrT = np.concatenate([sin_h, -sin_h], axis=0).astype(np.float32)

    i = np.arange(P)[:, None]
    j = np.arange(TQ)[None, :]
    maskadd = np.concatenate(
        [np.where(i + r * P <= j, 0.0, MASKVAL).astype(np.float32)
         for r in range(4)], axis=1
    )                                                     # [128, 4*512]
    ident = np.eye(P, dtype=np.float32)
    prot = np.roll(np.eye(P, dtype=np.float32), 64, axis=0)
    ones = np.ones((P, 1), dtype=np.float32)
    return np.ascontiguousarray(np.concatenate(
        [cosT, sinrT, maskadd, ident, prot, ones], axis=1))


def _tile_w(w):
    """[dims, HIDDEN] weight slice -> [P, KT, dims] pre-tiled layout."""
    return np.ascontiguousarray(
        w.T.reshape(KT, P, w.shape[0]).transpose(1, 0, 2))


def make_in_maps(hidden_states, wq, wk, wv, wo):
    hidden_states = np.asarray(hidden_states, dtype=np.float32)
    wq = np.asarray(wq, dtype=np.float32)
    wk = np.asarray(wk, dtype=np.float32)
    wv = np.asarray(wv, dtype=np.float32)
    wo = np.asarray(wo, dtype=np.float32)

    x2 = hidden_states.reshape(TOK, HIDDEN)
    # xt[blk, p, ko, t] = x2[blk*TQ + t, ko*P + p]
    xt = np.ascontiguousarray(
        x2.reshape(NBLK, TQ, KT, P).transpose(0, 3, 2, 1))
    consts = _host_consts()
    qscale = 1.0 / math.sqrt(HEAD_DIM)

    in_maps = []
    for c in range(NCORES):
        # packed wq|wk|wv slices along the output dim: [P, KT, 768]
        wqkv = np.concatenate([
            _tile_w(wq[c * HG:(c + 1) * HG] * qscale),
            _tile_w(wk[c * HEAD_DIM:(c + 1) * HEAD_DIM]),
            _tile_w(wv[c * HEAD_DIM:(c + 1) * HEAD_DIM]),
        ], axis=2)
        # wo row-slice [HIDDEN, HG] -> pre-tiled [P, HG//P, HIDDEN]:
        # wot[p, k, d] = wo[d, c*HG + k*128 + p]
        wo_sl = wo[:, c * HG:(c + 1) * HG]                # [HIDDEN, HG]
        wot = np.ascontiguousarray(
            wo_sl.T.reshape(HG // P, P, HIDDEN).transpose(1, 0, 2)
        ).astype(ml_dtypes.bfloat16)
        in_maps.append({
            "xt": xt,
            "wqkv": np.ascontiguousarray(wqkv),
            "wot": wot,
            "consts": consts,
        })
    return in_maps


def assemble_output(results):
    # outT per core: [P, MO, TOK] bf16 partials; out[d, t] = sum_c
    # part_c[d % 128, d // 128, t]
    acc = np.zeros((P, MO, TOK), dtype=np.float32)
    for c in range(NCORES):
        acc += results[c]["outT"].astype(np.float32)
    out = acc.transpose(1, 0, 2).reshape(HIDDEN, TOK)
    return np.ascontiguousarray(out.T).reshape(B, S, HIDDEN)


def kernel(hidden_states, wq, wk, wv, wo):
    nc = _get_module()
    in_maps = make_in_maps(hidden_states, wq, wk, wv, wo)
    trace = bool(int(os.environ.get("KERNEL_TRACE", "0")))
    res = bass_utils.run_bass_kernel_spmd(
        nc, in_maps, core_ids=list(range(NCORES)), trace=trace
    )
    if trace:
        kernel.last_results = res
    return assemble_output(res.results)


kernel.last_results = None


# revision 32
# speedup vs baseline: 29.9031x; 1.2310x over previous
"""InternLM3 attention block on 8 Trainium2 NeuronCores (Bass/Tile), v4.

Strategy (tensor-parallel over heads, collective-free):
  - 32 Q heads / 8 KV heads, head_dim 128.  Core c owns Q heads [4c,4c+4)
    and KV head c (one GQA group per core, so K/V never needs replication).
  - Per core, fused pipeline over 512-token blocks: QKV projection (fp32r
    matmuls, host-pretiled packed [p, ko, 768] layout so every DMA line is
    large and contiguous) -> RoPE (2 DVE mults off PSUM + PE rotation
    matmul + DVE add) -> causal flash-style attention in S^T orientation:
      scores^T = kT-tile.T @ q^T  (PSUM, fp32r width-floored at 256 since
      narrow fp32r runs at 1/4 PE rate), causal mask added via a bf16
      identity-matmul of a -60000 bias strip (exp -> exact 0), exp on ACT
      (bf16 out), denominator accumulated on PE via a bf16 ones-column
      matmul, PV in PSUM (bf16 operands, full rate at any width),
      normalize with DVE reciprocal + GpSimd partition_broadcast.
  - No collectives: instead of AllGather + column-sliced output projection,
    each core contracts its own 512 attention dims against its wo row-slice
    (wo[:, c*512:(c+1)*512]) producing a PARTIAL [4096, tok] output; the
    host sums the 8 partials.  Cores never synchronize, so per-core NEFF
    span is independent of launch skew and collective latency.
  - Inputs are packed into 4 device tensors (xt, wqkv, wot, consts) —
    per-argument dispatch overhead through the PJRT path is significant.
  - DMA is split across both HWDGE rings (sync: loads, scalar: stores).

Projection matmuls run fp32r (TF32-like, full PE rate at >=256 wide,
~1e-4 relative error); attention probabilities and the output projection
run bf16 (measured end-to-end error ~3e-3 against the fp32 reference).
"""

import math
import os
import sys

if "/opt/trn_rl_repo" not in sys.path:
    sys.path.insert(0, "/opt/trn_rl_repo")

import ml_dtypes
import numpy as np

import concourse.bass as bass
import concourse.mybir as mybir
import concourse.tile as tile
from concourse import bacc
from concourse import bass_utils

# ---- problem constants (hardcoded per harness contract) ----
HIDDEN = 4096
N_HEADS = 32
N_KV_HEADS = 8
HEAD_DIM = 128
ROPE_THETA = 10000.0
B, S = 2, 2048
NCORES = 8

P = 128
TQ = 512                      # token block
NB = S // TQ                  # 4 blocks per batch
KT = HIDDEN // P              # 32 contraction tiles
QH = N_HEADS // NCORES        # 4 q-heads per core
HG = QH * HEAD_DIM            # 512 = head-group width per core
WKV = HG + 2 * HEAD_DIM       # 768 = packed wq|wk|wv width
NBLK = B * NB                 # 8 token blocks
TOK = B * S                   # 4096 tokens
KB = 8                        # k-tiles per x DMA (1MB bf16 chunks)
MO = HIDDEN // P              # 32 output-dim tiles in the partial out-proj
MASKVAL = -60000.0            # exp(score + MASKVAL) == 0 exactly

# packed consts layout (f32 columns)
C_COS = 0
C_SINR = C_COS + S
C_MASK = C_SINR + S
C_ID = C_MASK + 4 * TQ
C_PROT = C_ID + P
C_ONES = C_PROT + P
C_COLS = C_ONES + 1

f32 = mybir.dt.float32
f32r = mybir.dt.float32r
bf16 = mybir.dt.bfloat16


def _build_module():
    nc = bacc.Bacc("TRN2", target_bir_lowering=False, debug=False)

    xt = nc.dram_tensor("xt", [NBLK, P, KT, TQ], bf16,
                        kind="ExternalInput").ap()
    wqkv = nc.dram_tensor("wqkv", [P, KT, WKV], bf16,
                          kind="ExternalInput").ap()
    # wo row-slice for this core, pre-tiled: [p, k(4), HIDDEN]
    wot = nc.dram_tensor("wot", [P, HG // P, HIDDEN], bf16,
                         kind="ExternalInput").ap()
    consts = nc.dram_tensor("consts", [P, C_COLS], f32,
                            kind="ExternalInput").ap()
    # partial output projection: out[p, m, t] = out-dim (m*128+p), token t
    outT = nc.dram_tensor("outT", [P, MO, TOK], bf16,
                          kind="ExternalOutput").ap()

    ao_dram = [
        nc.dram_tensor(f"ao{i}", [HG, TQ], bf16, kind="Internal").ap()
        for i in range(NBLK)
    ]

    with tile.TileContext(nc) as tc:
        _body(tc, nc, xt, wqkv, wot, consts, outT, ao_dram)
    nc.compile()
    return nc


def _body(tc, nc, xt, wqkv, wot, consts, outT, ao_dram):
    AF = mybir.ActivationFunctionType
    OP = mybir.AluOpType

    with (
        tc.tile_pool(name="wpool", bufs=1) as wpool,
        tc.tile_pool(name="xpool", bufs=3) as xpool,
        tc.tile_pool(name="kvpool", bufs=1) as kvpool,
        tc.tile_pool(name="qpool", bufs=1) as qpool,
        tc.tile_pool(name="rtmp", bufs=2) as rtmp,
        tc.tile_pool(name="epool", bufs=4) as epool,
        tc.tile_pool(name="aux", bufs=2) as aux,
        tc.tile_pool(name="atpool", bufs=3) as atpool,
        tc.tile_pool(name="obpool", bufs=3) as obpool,
        tc.tile_pool(name="pq", bufs=1, space="PSUM") as pq,
        tc.tile_pool(name="pk", bufs=1, space="PSUM") as pk,
        tc.tile_pool(name="ppv", bufs=1, space="PSUM") as ppv,
        tc.tile_pool(name="pst", bufs=2, space="PSUM") as pst,
    ):
        # ---- resident weights / constants (packed, pre-tiled) ----
        # DMA order follows the critical path (HWDGE drains FIFO): the
        # k-tiles the first matmuls need, then block 0's x tiles
        # interleaved with the rest of wqkv; the packed consts are issued
        # inside block 0 right before their first consumers.
        wqkv_sb = wpool.tile([P, KT, WKV], bf16, tag="wqkv")
        c_sb = wpool.tile([P, C_COLS], f32, tag="consts")
        mask_sb = wpool.tile([P, 4 * TQ], bf16, tag="mask")
        idb_sb = wpool.tile([P, P], bf16, tag="identb")
        ones_sb = wpool.tile([P, 1], bf16, tag="ones")
        # fp32r matmul operands must be PRODUCED as fp32r (BIR verifier),
        # so prot gets its own tile, DMA'd with the bitcast on the DRAM side
        prot_sb = wpool.tile([P, P], f32r, tag="prot")
        # wo is resident from phase 1 (bf16 x/wqkv freed the SBUF) so the
        # phase-2 output projection starts without waiting on its load
        wo_sb = wpool.tile([P, HG // P, HIDDEN], bf16, tag="wo")

        cos_sb = c_sb[:, C_COS:C_COS + S]
        sinr_sb = c_sb[:, C_SINR:C_SINR + S]
        id_sb = c_sb[:, C_ID:C_ID + P]

        nc.sync.dma_start(wqkv_sb[:, 0:16, :], wqkv[:, 0:16, :])
        x_pre = []
        for i in range(3):
            xtile = xpool.tile([P, KB, TQ], bf16, tag="x", name=f"xpre{i}")
            nc.sync.dma_start(xtile[:], xt[0, :, i * KB:(i + 1) * KB, :])
            x_pre.append(xtile)
            if i < 2:
                nc.sync.dma_start(
                    wqkv_sb[:, 16 + i * 8:16 + (i + 1) * 8, :],
                    wqkv[:, 16 + i * 8:16 + (i + 1) * 8, :])

        def _const_setup():
            nc.sync.dma_start(c_sb[:], consts)
            nc.sync.dma_start(
                prot_sb[:], consts.bitcast(f32r)[:, C_PROT:C_PROT + P])
            # bf16 working copies (bf16 matmul operands run 1 cycle/row on
            # PE at any width; narrow fp32r would run at 1/4 rate)
            nc.vector.tensor_copy(mask_sb[:],
                                  c_sb[:, C_MASK:C_MASK + 4 * TQ])
            nc.vector.tensor_copy(idb_sb[:], id_sb)
            nc.vector.tensor_copy(ones_sb[:], c_sb[:, C_ONES:C_ONES + 1])
            nc.sync.dma_start(wo_sb[:], wot)

        def rope(dst_f32r, src_ps, n):
            """dst = src*cos + rotate_half(src)*sin for token block n.

            src_ps is a [P, TQ] fp32 PSUM AP (projection output); the two
            DVE mults double as the PSUM evacuation.  The half-rotation
            runs on PE via the Prot permutation matmul."""
            cos_blk = cos_sb[:, n * TQ:(n + 1) * TQ]
            sinr_blk = sinr_sb[:, n * TQ:(n + 1) * TQ]
            qcos = rtmp.tile([P, TQ], f32, tag="qcos")
            nc.vector.tensor_tensor(qcos[:], src_ps, cos_blk, OP.mult)
            qsin = rtmp.tile([P, TQ], f32r, tag="qsin")
            nc.vector.tensor_tensor(qsin[:], src_ps, sinr_blk, OP.mult)
            rot_ps = pst.tile([P, TQ], f32, tag="st", name="rot")
            nc.tensor.matmul(rot_ps[:], prot_sb[:], qsin[:],
                             start=True, stop=True)
            nc.vector.tensor_tensor(dst_f32r, qcos[:], rot_ps[:], OP.add)

        for b in range(B):
            kT_cache = kvpool.tile([P, S], f32r, tag="kT")
            v_cache = kvpool.tile([P, S // P, HEAD_DIM], bf16, tag="v")
            for n in range(NB):
                blk = b * NB + n
                # ---------- QKV projection for this token block ----------
                q_ps = [
                    pq.tile([P, TQ], f32, tag=f"q{j}", name=f"qps{j}")
                    for j in range(QH)
                ]
                k_ps = pk.tile([P, TQ], f32, tag="kk", name="kps")
                v_ps = ppv.tile([P, TQ], f32, tag="pv", name="vps")
                for k8 in range(KT // KB):
                    if blk == 0 and k8 < 3:
                        x_t = x_pre[k8]
                    else:
                        x_t = xpool.tile([P, KB, TQ], bf16, tag="x")
                        nc.sync.dma_start(
                            x_t[:], xt[blk, :, k8 * KB:(k8 + 1) * KB, :])
                    for kk in range(KB):
                        k = k8 * KB + kk
                        st = dict(start=(k == 0), stop=(k == KT - 1))
                        for j in range(QH):
                            nc.tensor.matmul(
                                q_ps[j][:],
                                wqkv_sb[:, k, j * P:(j + 1) * P],
                                x_t[:, kk, :], **st
                            )
                        nc.tensor.matmul(
                            k_ps[:], wqkv_sb[:, k, HG:HG + HEAD_DIM],
                            x_t[:, kk, :], **st)
                        nc.tensor.matmul(
                            v_ps[:],
                            wqkv_sb[:, k, HG + HEAD_DIM:HG + 2 * HEAD_DIM],
                            x_t[:, kk, :], **st)

                if blk == 0:
                    _const_setup()
                # ---------- RoPE (also evacuates q/k PSUM banks) ----------
                qT_sb = qpool.tile([P, QH, TQ], f32r, tag="q")
                for j in range(QH):
                    rope(qT_sb[:, j, :], q_ps[j][:], n)
                rope(kT_cache[:, n * TQ:(n + 1) * TQ], k_ps[:], n)

                # ---------- V: evacuate + transpose to [tok, dim] ----------
                vT_sb = rtmp.tile([P, TQ], f32, tag="vtsb")
                nc.scalar.copy(vT_sb[:], v_ps[:])
                for j in range(4):
                    tp = pst.tile([P, TQ], f32, tag="st", name="vtp")
                    nc.tensor.transpose(
                        tp[:, :P], vT_sb[:, j * P:(j + 1) * P], id_sb
                    )
                    nc.vector.tensor_copy(
                        v_cache[:, n * 4 + j, :], tp[:, :P]
                    )

                # ---------- attention, one GQA head at a time ----------
                ntk = (n + 1) * (TQ // P)
                for h in range(QH):
                    pv_ps = ppv.tile([P, TQ], f32, tag="pv", name="pvps")
                    dn_ps = pk.tile([P, TQ], f32, tag="kk", name="dnps")
                    qr = qT_sb[:, h, :]
                    for t in range(ntk):
                        diag = t >= ntk - 4
                        # diag position r: columns < 128r are fully masked
                        # (zero contribution) -> restrict every op to the
                        # live range [c0, TQ); bit-identical, less stream.
                        # The f32r score matmul floors its width at 256
                        # (f32r below 256 wide runs at 1/4 PE rate, so 256
                        # columns are cheaper than 128); extra columns land
                        # in PSUM but are never read.
                        r = t - (ntk - 4)
                        c0 = P * r if diag else 0
                        lo = min(c0, TQ - 256) if diag else 0
                        st_ps = pst.tile([P, TQ], f32, tag="st", name="stps")
                        nc.tensor.matmul(
                            st_ps[:, lo:], kT_cache[:, t * P:(t + 1) * P],
                            qr[:, lo:],
                            start=True, stop=not diag,
                        )
                        if diag:
                            # the triangular boundary lives in one strip
                            nc.tensor.matmul(
                                st_ps[:, c0:c0 + P], idb_sb[:],
                                mask_sb[:, r * TQ + c0:r * TQ + c0 + P],
                                start=False, stop=True,
                                skip_group_check=True,
                            )
                        es = epool.tile([P, TQ], bf16, tag="es")
                        nc.scalar.activation(es[:, c0:], st_ps[:, c0:],
                                             AF.Exp)
                        nc.tensor.matmul(
                            dn_ps[:1, c0:], ones_sb[:], es[:, c0:],
                            start=(t == 0), stop=(t == ntk - 1),
                            skip_group_check=True,
                        )
                        nc.tensor.matmul(
                            pv_ps[:, c0:], v_cache[:, t, :], es[:, c0:],
                            start=(t == 0), stop=(t == ntk - 1),
                            skip_group_check=True,
                        )
                    # normalize: 1/denominator broadcast over partitions
                    # (approx_fast: ~51 ULP, 5x faster than iterative divide;
                    # denominators are sums of exps, well inside safe range).
                    # The broadcast runs on the otherwise-idle GpSimd engine
                    # (no collectives in this kernel to serialize behind).
                    rec = aux.tile([1, TQ], f32, tag="rec")
                    nc.vector.reciprocal_approx_fast(rec[:], dn_ps[:1, :])
                    pv_sb = aux.tile([P, TQ], f32, tag="pvs")
                    nc.scalar.copy(pv_sb[:], pv_ps[:])
                    bcb = aux.tile([P, TQ], f32, tag="bcb")
                    nc.gpsimd.partition_broadcast(bcb[:], rec[:1, :],
                                                  channels=P)
                    ao = aux.tile([P, TQ], bf16, tag="ao")
                    nc.vector.tensor_tensor(ao[:], pv_sb[:], bcb[:], OP.mult)
                    nc.scalar.dma_start(
                        ao_dram[blk][h * P:(h + 1) * P, :], ao[:]
                    )

        # -------- phase 2: partial output projection (no collective) --
        # out[m*128+p, t] = sum_k wo[m*128+p, cHG+k*128+j] * ao[k*128+j, t]
        # contraction over this core's 512 attention dims only; the host
        # sums the 8 per-core partials.  Accumulators rotate over all 8
        # PSUM banks (reusing the phase-1 pools' tags).
        def op_tile(i, name):
            s = i % 8
            if s < 4:
                return pq.tile([P, TQ], f32, tag=f"q{s}", name=name)
            if s == 4:
                return pk.tile([P, TQ], f32, tag="kk", name=name)
            if s == 5:
                return ppv.tile([P, TQ], f32, tag="pv", name=name)
            return pst.tile([P, TQ], f32, tag="st", name=name)

        for ch in range(NBLK):
            ao_r = ao_dram[ch].rearrange("(k p) t -> p k t", p=P)
            at = atpool.tile([P, HG // P, TQ], bf16, tag="at",
                             name=f"at{ch}")
            nc.sync.dma_start(at[:], ao_r)
            for m2 in range(MO // 4):
                op_ps = [
                    op_tile(m2 * 4 + j, f"op{ch}_{m2}_{j}")
                    for j in range(4)
                ]
                ob = obpool.tile([P, 4, TQ], bf16, tag="ob")
                for j in range(4):
                    m = m2 * 4 + j
                    for k in range(HG // P):
                        nc.tensor.matmul(
                            op_ps[j][:],
                            wo_sb[:, k, m * P:(m + 1) * P],
                            at[:, k, :],
                            start=(k == 0), stop=(k == HG // P - 1),
                        )
                    nc.vector.tensor_copy(ob[:, j, :], op_ps[j][:])
                nc.scalar.dma_start(
                    outT[:, m2 * 4:(m2 + 1) * 4, ch * TQ:(ch + 1) * TQ],
                    ob[:]
                )


_NC_CACHE = None


def _get_module():
    global _NC_CACHE
    if _NC_CACHE is None:
        _NC_CACHE = _build_module()
    return _NC_CACHE


def _host_consts():
    inv_freq = 1.0 / (ROPE_THETA ** (np.arange(0, HEAD_DIM, 2,
                                               dtype=np.float32) / HEAD_DIM))
    t = np.arange(S, dtype=np.float32)
    freqs = np.outer(t, inv_freq).astype(np.float32)      # [S, 64]
    cos_h = np.cos(freqs).T                               # [64, S]
    sin_h = np.sin(freqs).T
    cosT = np.concatenate([cos_h, cos_h], axis=0).astype(np.float32)
    # ssin = [-sin; sin];  sinrot[r] = ssin[(r+64)%128] = [sin; -sin]
    sinrT = np.concatenate([sin_h, -sin_h], axis=0).astype(np.float32)

    i = np.arange(P)[:, None]
    j = np.arange(TQ)[None, :]
    maskadd = np.concatenate(
        [np.where(i + r * P <= j, 0.0, MASKVAL).astype(np.float32)
         for r in range(4)], axis=1
    )                                                     # [128, 4*512]
    ident = np.eye(P, dtype=np.float32)
    prot = np.roll(np.eye(P, dtype=np.float32), 64, axis=0)
    ones = np.ones((P, 1), dtype=np.float32)
    return np.ascontiguousarray(np.concatenate(
        [cosT, sinrT, maskadd, ident, prot, ones], axis=1))


def _tile_w(w):
    """[dims, HIDDEN] weight slice -> [P, KT, dims] pre-tiled layout."""
    return np.ascontiguousarray(
        w.T.reshape(KT, P, w.shape[0]).transpose(1, 0, 2))


def make_in_maps(hidden_states, wq, wk, wv, wo):
    hidden_states = np.asarray(hidden_states, dtype=np.float32)
    wq = np.asarray(wq, dtype=np.float32)
    wk = np.asarray(wk, dtype=np.float32)
    wv = np.asarray(wv, dtype=np.float32)
    wo = np.asarray(wo, dtype=np.float32)

    x2 = hidden_states.reshape(TOK, HIDDEN)
    # xt[blk, p, ko, t] = x2[blk*TQ + t, ko*P + p]
    xt = np.ascontiguousarray(
        x2.reshape(NBLK, TQ, KT, P).transpose(0, 3, 2, 1)
    ).astype(ml_dtypes.bfloat16)
    consts = _host_consts()
    qscale = 1.0 / math.sqrt(HEAD_DIM)

    in_maps = []
    for c in range(NCORES):
        # packed wq|wk|wv slices along the output dim: [P, KT, 768]
        wqkv = np.concatenate([
            _tile_w(wq[c * HG:(c + 1) * HG] * qscale),
            _tile_w(wk[c * HEAD_DIM:(c + 1) * HEAD_DIM]),
            _tile_w(wv[c * HEAD_DIM:(c + 1) * HEAD_DIM]),
        ], axis=2).astype(ml_dtypes.bfloat16)
        # wo row-slice [HIDDEN, HG] -> pre-tiled [P, HG//P, HIDDEN]:
        # wot[p, k, d] = wo[d, c*HG + k*128 + p]
        wo_sl = wo[:, c * HG:(c + 1) * HG]                # [HIDDEN, HG]
        wot = np.ascontiguousarray(
            wo_sl.T.reshape(HG // P, P, HIDDEN).transpose(1, 0, 2)
        ).astype(ml_dtypes.bfloat16)
        in_maps.append({
            "xt": xt,
            "wqkv": np.ascontiguousarray(wqkv),
            "wot": wot,
            "consts": consts,
        })
    return in_maps


def _bf16_to_f32(a):
    """Exact bf16->f32 widening via bit ops (much faster than ml_dtypes)."""
    u = np.asarray(a).view(np.uint16).astype(np.uint32) << 16
    return u.view(np.float32)


def assemble_output(results):
    # outT per core: [P, MO, TOK] bf16 partials; out[d, t] = sum_c
    # part_c[d % 128, d // 128, t]
    acc = np.zeros((P, MO, TOK), dtype=np.float32)
    for c in range(NCORES):
        acc += _bf16_to_f32(results[c]["outT"])
    out = acc.transpose(1, 0, 2).reshape(HIDDEN, TOK)
    return np.ascontiguousarray(out.T).reshape(B, S, HIDDEN)


_RUNNER = None


def _get_runner():
    """Cached jit executable + device-resident zero output buffers, so
    repeat kernel() calls skip retracing/recompiling the dispatch fn."""
    global _RUNNER
    if _RUNNER is not None:
        return _RUNNER
    import jax
    from jax.experimental.shard_map import shard_map
    from jax.sharding import Mesh, NamedSharding, PartitionSpec
    import concourse.mybir as mybir
    from concourse import bass2jax

    nc = _get_module()
    bass2jax.install_neuronx_cc_hook()
    part_name = (nc.partition_id_tensor.name
                 if nc.partition_id_tensor else None)
    in_names, out_names, out_avals, zero_outs = [], [], [], []
    for alloc in nc.m.functions[0].allocations:
        if not isinstance(alloc, mybir.MemoryLocationSet):
            continue
        name = alloc.memorylocations[0].name
        if alloc.kind == "ExternalInput":
            if name != part_name:
                in_names.append(name)
        elif alloc.kind == "ExternalOutput":
            out_names.append(name)
            shape = tuple(alloc.tensor_shape)
            dtype = mybir.dt.np(alloc.dtype)
            out_avals.append(jax.core.ShapedArray(shape, dtype))
            zero_outs.append(np.zeros(shape, dtype))
    n_params = len(in_names)

    def _bodyfn(*args):
        operands = list(args)
        all_in = in_names + out_names
        if part_name is not None:
            operands.append(bass2jax.partition_id_tensor())
            all_in = all_in + [part_name]
        outs = bass2jax._bass_exec_p.bind(
            *operands,
            out_avals=tuple(out_avals),
            in_names=tuple(all_in),
            out_names=tuple(out_names),
            lowering_input_output_aliases=(),
            sim_require_finite=True,
            sim_require_nnan=True,
            nc=nc,
        )
        return tuple(outs)

    devices = jax.devices()[:NCORES]
    mesh = Mesh(np.asarray(devices), ("core",))
    # xt and consts are identical on every core: ship them replicated
    # (one host->device transfer instead of eight across the slow tunnel)
    replicated = {"xt", "consts"}
    in_specs = tuple(
        PartitionSpec() if nm in replicated else PartitionSpec("core")
        for nm in in_names
    ) + (PartitionSpec("core"),) * len(out_names)
    fn = jax.jit(
        shard_map(_bodyfn, mesh=mesh, in_specs=in_specs,
                  out_specs=(PartitionSpec("core"),) * len(out_names),
                  check_rep=False),
        keep_unused=True,
    )
    sh = NamedSharding(mesh, PartitionSpec("core"))
    sh_rep = NamedSharding(mesh, PartitionSpec())
    dev_zero = [
        jax.device_put(np.concatenate([z] * NCORES, axis=0), sh)
        for z in zero_outs
    ]
    _RUNNER = (fn, sh, sh_rep, replicated, in_names[:n_params], out_names,
               out_avals, dev_zero)
    return _RUNNER


def kernel(hidden_states, wq, wk, wv, wo):
    trace = bool(int(os.environ.get("KERNEL_TRACE", "0")))
    in_maps = make_in_maps(hidden_states, wq, wk, wv, wo)
    if trace:
        nc = _get_module()
        res = bass_utils.run_bass_kernel_spmd(
            nc, in_maps, core_ids=list(range(NCORES)), trace=True
        )
        kernel.last_results = res
        return assemble_output(res.results)

    import jax
    (fn, sh, sh_rep, replicated, in_names, out_names, out_avals,
     dev_zero) = _get_runner()
    dev_in = []
    for nm in in_names:
        if nm in replicated:
            dev_in.append(jax.device_put(np.asarray(in_maps[0][nm]),
                                         sh_rep))
        else:
            dev_in.append(jax.device_put(
                np.concatenate([np.asarray(in_maps[c][nm]) for c in
                                range(NCORES)], axis=0), sh))
    outs = fn(*dev_in, *dev_zero)
    results = [
        {nm: np.asarray(outs[i]).reshape(NCORES, *out_avals[i].shape)[c]
         for i, nm in enumerate(out_names)}
        for c in range(NCORES)
    ]
    return assemble_output(results)


kernel.last_results = None
